# revision 1
# baseline (speedup 1.0000x reference)
"""CLIP-Adapter loss kernel for 8 trn2 NeuronCores (data-parallel over batch).

Math (reference):
    h        = relu(img @ w1 + b1)
    adapted  = relu(h @ w2 + b2)
    x        = alpha*img + (1-alpha)*adapted
    sim      = (x @ txt) * exp(logit_scale); sim /= ||sim||_row (twice)
    loss     = -mean(log_softmax(sim / t)[i, tgt_i])
    acc      = sum(argmax_row(rownorm(x @ txt)) == tgt)

Reformulation used here (exact up to fp rounding):
  * exp(logit_scale) and the second row-normalization cancel mathematically.
  * Let raw = x @ txt, u_i = 1/(t*||raw_i||). Then
        nll_i  = log(sum_j exp(raw_ij*u_i)) - raw_i[tgt_i]*u_i
        acc_i  = (raw_i[tgt_i] == max_j raw_ij)
  * We compute raw' = raw/(1-alpha) instead (positive row-constant scale:
    cancels in u*raw and preserves argmax):
        A2T  = (alpha/(1-alpha)) * img_shard^T      (host prep)
        w2s  = ((1-alpha)/alpha) * w2               (host prep)
        h''  = relu(A2T^T-matmul w1 + s*b1)  = s*h  (s = alpha/(1-alpha))
        y    = h'' @ w2s                      = h @ w2
        x'^T = relu(y^T) + A2T                (one fused DVE op; b2 == 0)
        raw' = x'^T^T @ txt                   = raw/(1-alpha)
Each core outputs [sum_i nll_i, sum_i acc_i]; host combines the 8 partials.
"""

import math
import numpy as np

import concourse.bass as bass
import concourse.bacc as bacc
import concourse.tile as tile
import concourse.hw_specs as _hw_specs

# All activations used here (Relu/Square/Ln/Exp/Copy) live in the single
# table set natural_log_exp_and_others. The default chooser alternates
# between sets (Exp->exp_and_others, Ln->natural_log), inserting an
# ~2.7us ACT table load per switch -- ~64 loads per pass. Restrict the
# chooser to the one set that covers everything.
_orig_get_tables = _hw_specs.get_activation_tables


def _only_lnexp_tables(arch):
    tables = _orig_get_tables(arch)
    name = "natural_log_exp_and_others"
    if name not in tables:
        return tables
    mine = {
        mybir.ActivationFunctionType.Relu,
        mybir.ActivationFunctionType.Square,
        mybir.ActivationFunctionType.Ln,
        mybir.ActivationFunctionType.Exp,
        mybir.ActivationFunctionType.Copy,
        mybir.ActivationFunctionType.Identity,
    }
    assert mine <= tables[name]
    # Positions are the act_func_set_id walrus uses -- keep every set in
    # place, just remove my functions from the other sets so the chooser
    # always lands on the combined set.
    return {
        nm: (fns if nm == name else (fns - mine))
        for nm, fns in tables.items()
    }


bacc.get_activation_tables = _only_lnexp_tables
from concourse import mybir
from concourse.bass_utils import run_bass_kernel_spmd

F32 = mybir.dt.float32
F32R = mybir.dt.float32r
BF16 = mybir.dt.bfloat16
AF = mybir.ActivationFunctionType
ALU = mybir.AluOpType

B, D, H, N = 32768, 512, 128, 1000
NCORES = 8
R = B // NCORES          # rows per core
KC = D // 128            # k-chunks (4)
NT = R // 128            # row tiles per core (32)
NG = R // 512            # row groups per core (8)
N0, N1 = 512, N - 512    # logits split per PSUM bank


def _r(ap):
    return ap.bitcast(F32R)


def build_nc(t_val: float, b1s_np: np.ndarray, b2_np: np.ndarray, repeat: int = 1,
             loop: int = 0, ablate: frozenset = frozenset()):
    """Build the per-core Bass program (identical on all 8 cores)."""
    b2_zero = not np.any(b2_np)
    nc = bacc.Bacc("TRN2", target_bir_lowering=False)

    a2t = nc.declare_dram_parameter("a2t", [D, R], BF16, isOutput=False)
    txt = nc.declare_dram_parameter("txt", [D, N], BF16, isOutput=False)
    w1 = nc.declare_dram_parameter("w1", [D, H], BF16, isOutput=False)
    w2s = nc.declare_dram_parameter("w2s", [H, D], BF16, isOutput=False)
    b1s = nc.declare_dram_parameter("b1s", [H, 1], F32, isOutput=False)
    b2p = (None if b2_zero else
           nc.declare_dram_parameter("b2p", [128, KC], F32, isOutput=False))
    txtg = nc.declare_dram_parameter("txtg", [D, R], BF16, isOutput=False)
    identd = nc.declare_dram_parameter("identd", [128, 128], F32, isOutput=False)
    outp = nc.declare_dram_parameter("out", [1, 2], F32, isOutput=True)

    a2t_v = a2t[:].rearrange("(k p) r -> p k r", p=128)
    txtg_v = txtg[:].rearrange("(k p) r -> p k r", p=128)
    txt_v = txt[:].rearrange("(k p) n -> p k n", p=128)
    w1_v = w1[:].rearrange("(k p) h -> p k h", p=128)

    with tile.TileContext(nc) as tc:
        with (
            tc.tile_pool(name="singles", bufs=1) as singles,
            tc.tile_pool(name="aT", bufs=4) as aT_pool,
            tc.tile_pool(name="xT", bufs=4) as xT_pool,
            tc.tile_pool(name="hsb", bufs=3) as h_pool,
            tc.tile_pool(name="junk", bufs=1) as junk_pool,
            tc.tile_pool(name="ps_misc", bufs=1, space="PSUM") as ps_misc,
            tc.tile_pool(name="ps_dg", bufs=1, space="PSUM") as ps_dg,
            tc.tile_pool(name="ps_y", bufs=2, space="PSUM") as ps_y,
            tc.tile_pool(name="ps_raw", bufs=2, space="PSUM") as ps_raw,
        ):
            # ---- resident constants -------------------------------------
            txt_sb = singles.tile([128, KC, N], BF16)
            nc.sync.dma_start(out=txt_sb, in_=txt_v)
            w1_sb = singles.tile([128, KC, H], BF16)
            nc.sync.dma_start(out=w1_sb, in_=w1_v)
            w2_sb = singles.tile([128, D], BF16)
            nc.sync.dma_start(out=w2_sb, in_=w2s[:])
            b1_sb = singles.tile([128, 1], F32)
            nc.sync.dma_start(out=b1_sb, in_=b1s[:])
            ident_sb = singles.tile([128, 128], F32)
            nc.sync.dma_start(out=ident_sb, in_=identd[:])
            if not b2_zero:
                b2_sb = singles.tile([128, KC], F32)
                nc.sync.dma_start(out=b2_sb, in_=b2p[:])

            ones_sb = singles.tile([128, 1], F32)
            nc.vector.memset(ones_sb, 1.0)

            # per-row statistics, one column per row-tile
            SS = singles.tile([128, NT], F32)    # sum(raw^2)
            LNS = singles.tile([128, NT], F32)   # ln(SS)
            INV = singles.tile([128, NT], F32)   # 1/(t*sqrt(SS))
            SE = singles.tile([128, NT], F32)    # sum(exp(raw*inv))
            MX = singles.tile([128, NT], F32)    # max(raw)
            PK = singles.tile([128, NT], F32)    # raw[tgt]
            LSE = singles.tile([128, NT], F32)   # ln(SE)
            PKU = singles.tile([128, NT], F32)   # PK*INV
            J32 = singles.tile([128, NT], F32)   # LSE - PKU
            EQ32 = singles.tile([128, NT], F32)  # PK == MX flags
            RED = singles.tile([128, 2], F32)    # [nll partial, acc partial]

            junkA = junk_pool.tile([128, N], F32)  # ACT full-size out sink
            junkD = junk_pool.tile([128, N], F32)  # DVE full-size out sink
            J512 = junk_pool.tile([128, 4, 128], F32)  # diag extract scratch

            for _nm, _tile in (("pick", PK), ("max", MX), ("sq", SS),
                               ("exp", SE), ("inv", INV)):
                if _nm in ablate:
                    nc.vector.memset(_tile, 1.0)
            if "sq" in ablate:
                nc.vector.memset(LNS, 1.0)

            ln_inv_t = float(-math.log(t_val))   # bias so exp gives 1/t factor

            import contextlib
            loop_ctx = (tc.For_i(0, loop, 1,
                                 hint_engines=(mybir.EngineType.PE,
                                               mybir.EngineType.Activation,
                                               mybir.EngineType.DVE))
                        if loop else contextlib.nullcontext())
            with loop_ctx:
             for _rep in range(repeat):
              for g in range(NG):
                aT = aT_pool.tile([128, KC, 512], BF16)
                nc.sync.dma_start(out=aT, in_=a2t_v[:, :, g * 512:(g + 1) * 512])
                tgT = aT_pool.tile([128, KC, 512], BF16, tag="tgT")
                nc.sync.dma_start(out=tgT, in_=txtg_v[:, :, g * 512:(g + 1) * 512])

                # mm1: h''^T[128H, 512 rows] accumulated over KC chunks
                hps = ps_misc.tile([128, 512], F32, tag="misc")
                for k in range(KC):
                    nc.tensor.matmul(
                        hps, w1_sb[:, k, :], aT[:, k, :],
                        start=(k == 0), stop=(k == KC - 1),
                    )
                h_sb = h_pool.tile([128, 512], BF16)
                nc.vector.tensor_scalar(
                    out=h_sb, in0=hps, scalar1=b1_sb, scalar2=0.0,
                    op0=ALU.add, op1=ALU.max,
                )

                # mm2 + fused relu/blend: x'^T = relu(y(+b2)) + A2T
                xT = xT_pool.tile([128, KC, 512], BF16)
                for k in range(KC):
                    yps = ps_y.tile([128, 512], F32)
                    nc.tensor.matmul(
                        yps, w2_sb[:, k * 128:(k + 1) * 128], h_sb,
                        start=True, stop=True,
                    )
                    if "blend" in ablate:
                        nc.scalar.activation(xT[:, k, :], yps, AF.Relu)
                    elif b2_zero:
                        nc.vector.scalar_tensor_tensor(
                            out=xT[:, k, :], in0=yps, scalar=0.0,
                            in1=aT[:, k, :], op0=ALU.max, op1=ALU.add,
                        )
                    else:
                        u_sb = h_pool.tile([128, 512], BF16, tag="u")
                        nc.scalar.activation(
                            u_sb, yps, AF.Relu,
                            bias=b2_sb[:, k:k + 1], scale=1.0,
                        )
                        nc.vector.tensor_add(xT[:, k, :], u_sb, aT[:, k, :])

                # mm3 + per-row stats for the 4 row-tiles of this group
                if "pick" not in ablate:
                    dps_g = ps_dg.tile([128, 4, 128], F32, name="dps_g")
                for j in range(4):
                    t_idx = g * 4 + j
                    raw = ps_raw.tile([128, N], F32)
                    for k in range(KC):
                        lhsT = xT[:, k, j * 128:(j + 1) * 128]
                        nc.tensor.matmul(
                            raw[:, 0:N0], lhsT, txt_sb[:, k, 0:N0],
                            start=(k == 0), stop=(k == KC - 1),
                        )
                        nc.tensor.matmul(
                            raw[:, N0:N], lhsT, txt_sb[:, k, N0:N],
                            start=(k == 0), stop=(k == KC - 1),
                        )
                        if "pick" not in ablate:
                            nc.tensor.matmul(
                                dps_g[:, j, :], lhsT,
                                tgT[:, k, j * 128:(j + 1) * 128],
                                start=(k == 0), stop=(k == KC - 1),
                            )

                    tc_ = t_idx  # column in stat tiles
                    # row max -> MX  (DVE, emitted first so DVE starts the
                    # moment raw lands)
                    if "max" not in ablate:
                     nc.vector.tensor_reduce(
                        MX[:, tc_:tc_ + 1], raw, mybir.AxisListType.X, ALU.max,
                     )
                    # sum of squares -> SS  (ACT)
                    if "sq" not in ablate:
                     nc.scalar.activation(
                        junkA, raw, AF.Square,
                        accum_out=SS[:, tc_:tc_ + 1],
                     )
                    # inv = (1/t) * SS^-0.5 via ln/exp (same ACT table set)
                    if "inv" not in ablate:
                     nc.scalar.activation(
                        LNS[:, tc_:tc_ + 1], SS[:, tc_:tc_ + 1], AF.Ln,
                     )
                     nc.scalar.activation(
                        INV[:, tc_:tc_ + 1], LNS[:, tc_:tc_ + 1], AF.Exp,
                        scale=-0.5, bias=ln_inv_t,
                     )
                    # sum(exp(raw*inv)) -> SE  (ACT)
                    if "exp" not in ablate:
                     nc.scalar.activation(
                        junkA, raw, AF.Exp,
                        scale=INV[:, tc_:tc_ + 1],
                        accum_out=SE[:, tc_:tc_ + 1],
                     )

                # group-end: extract the 4 diagonals -> PK columns (DVE x2)
                if "pick" not in ablate:
                    nc.vector.tensor_mul(
                        J512, dps_g,
                        ident_sb[:].unsqueeze(1).broadcast_to([128, 4, 128]),
                    )
                    nc.vector.tensor_reduce(
                        PK[:, g * 4:(g + 1) * 4], J512,
                        mybir.AxisListType.X, ALU.add,
                    )


            # (emitted per group, appended after each group's j loop above)
            # ---- final reduction ----------------------------------------
            nc.scalar.activation(LSE, SE, AF.Ln)
            # PKU = PK*INV ; RED[:,0] = sum(LSE - PKU) ; RED[:,1] = sum(PK==MX)
            nc.vector.tensor_mul(PKU, PK, INV)
            nc.vector.tensor_tensor(J32, LSE, PKU, ALU.subtract)
            nc.vector.tensor_reduce(RED[:, 0:1], J32, mybir.AxisListType.X, ALU.add)
            nc.vector.tensor_tensor(EQ32, PK, MX, ALU.is_equal)
            nc.vector.tensor_reduce(RED[:, 1:2], EQ32, mybir.AxisListType.X, ALU.add)
            red_ps = ps_misc.tile([1, 2], F32, tag="misc", name="red_ps")
            nc.tensor.matmul(red_ps, ones_sb, RED, start=True, stop=True)
            red_sb = singles.tile([1, 2], F32)
            nc.scalar.copy(red_sb, red_ps)
            nc.sync.dma_start(out=outp[:], in_=red_sb)

    nc.compile()
    return nc


def _prep_inputs(inputs):
    A = np.ascontiguousarray(np.asarray(inputs["img_features"], dtype=np.float32))
    txt = np.ascontiguousarray(np.asarray(inputs["txt_features"], dtype=np.float32))
    w1 = np.ascontiguousarray(np.asarray(inputs["w1"], dtype=np.float32))
    b1 = np.asarray(inputs["b1"], dtype=np.float32).reshape(-1)
    w2 = np.ascontiguousarray(np.asarray(inputs["w2"], dtype=np.float32))
    b2 = np.asarray(inputs["b2"], dtype=np.float32).reshape(-1)
    alpha = float(np.asarray(inputs["alpha"]))
    tgt = np.asarray(inputs["target_ind"]).astype(np.int64)
    t_val = float(np.asarray(inputs["t"]))
    assert 0.0 < alpha < 1.0, f"alpha={alpha} not supported"
    assert A.shape == (B, D) and txt.shape == (D, N)

    import ml_dtypes
    bf16 = ml_dtypes.bfloat16
    s = alpha / (1.0 - alpha)
    w2s = np.ascontiguousarray((w2 / s).astype(bf16))
    b1s = (s * b1).astype(np.float32).reshape(H, 1)
    b2p = np.ascontiguousarray(b2.reshape(KC, 128).T).astype(np.float32)
    txt_bf = txt.astype(bf16)
    identd = np.eye(128, dtype=np.float32)
    in_maps = []
    for c in range(NCORES):
        sl = slice(c * R, (c + 1) * R)
        a2t = np.ascontiguousarray((s * A[sl]).T.astype(bf16))
        txtg = np.ascontiguousarray(txt[:, tgt[sl]].astype(bf16))
        in_maps.append({
            "a2t": a2t, "txt": txt_bf, "w1": w1.astype(bf16), "w2s": w2s,
            "b1s": b1s, "b2p": b2p, "txtg": txtg, "identd": identd,
        })
    return in_maps, b1s, b2, t_val


def _run(inputs, trace=False, **run_kwargs):
    in_maps, b1s, b2, t_val = _prep_inputs(inputs)
    nc = build_nc(t_val, b1s, b2)
    res = run_bass_kernel_spmd(
        nc, in_maps, list(range(NCORES)), trace=trace, **run_kwargs
    )
    nll = 0.0
    acc = 0.0
    for r in res.results:
        nll += float(r["out"][0, 0])
        acc += float(r["out"][0, 1])
    loss = np.float32(nll / B)
    return (loss, np.int32(round(acc))), res


def kernel(**inputs):
    out, _ = _run(inputs, trace=False)
    return out

